# revision 32
# baseline (speedup 1.0000x reference)
"""Multi-head self-attention (b=4, s=2048, d_model=1024, h=16, causal) on 8 trn2 cores.

Sharding: core c = (batch b = c//2, head-group g = c%2): 8 heads of one batch
per core, full QKV + causal attention + partial W_o projection on device; host
pre-transposes x/W slices and sums the two partial y's per batch (the W_o
all-reduce done at unshard time).

Single software-pipelined schedule: the ACT-bound exp stream of chunk j's
attention is the backbone; the PE-bound prep work for chunk j+1 (V and Q/K
projections) and the output projection of chunk j-1 are emitted as "filler
units" between attention i-tiles so the PE fills its exp-wait gaps and never
idles long enough for the HAM clock gate to re-throttle. x is loaded once per
chunk (shared by the V and Q/K projections), weight DMAs are issued in
first-use order (wv, x0, wq/wk, x1, wo), and the softmax-denominator
reciprocal+broadcast chain runs per pair right at its drain so the final
chunk's output projection has a short tail.

Scores/projections run as float32r (full PE rate at N>=256); the attn-weight /
V side runs bf16. Attention uses the transposed layout S^T[k,q] = K @ Q^T with
the two heads of a pair row-packed via tile_position (0,0)/(64,0) (concurrent
K=64 matmuls) into one [128,1024] psum tile -> single Exp per block. V carries
an appended ones column so denominators fall out of the attn@V matmul (row
64). Causality: block skip + column restriction + one triangular strip mask.
"""

from collections import deque

import ml_dtypes
import numpy as np

import concourse.bass as bass
import concourse.tile as tile
from concourse import bacc, mybir
from concourse.bass import ts
from concourse.bass_utils import run_bass_kernel_spmd

F32 = mybir.dt.float32
F32R = mybir.dt.float32r
BF16 = mybir.dt.bfloat16

B = 4
S = 2048
DM = 1024
DK = 64
N_CORES = 8
H = 8
PAIRS = 4
NKT = DM // 128   # 8 contraction tiles
NTT = S // 128    # 16 token tiles
NQC = S // 512    # 4 query chunks
AUG = DK + 1      # 65


def _kernel_body(ctx, tc):
    nc = tc.nc
    # All inputs arrive pre-tiled on the host to [128, ...] partition-major
    # layouts so every load is a single fully-contiguous DMA.
    xT = nc.dram_tensor("xT", [128, NQC * NKT * 512], BF16,
                        kind="ExternalInput").ap()
    wqT = nc.dram_tensor("wqT", [128, NKT * H * DK], BF16,
                         kind="ExternalInput").ap()
    wkT = nc.dram_tensor("wkT", [128, NKT * H * DK], BF16,
                         kind="ExternalInput").ap()
    wvT = nc.dram_tensor("wvT", [128, NKT * H * DK], BF16,
                         kind="ExternalInput").ap()
    woT = nc.dram_tensor("woT", [128, PAIRS * DM], BF16,
                         kind="ExternalInput").ap()
    tri = nc.dram_tensor("tri", [128, 128], F32, kind="ExternalInput").ap()
    y = nc.dram_tensor("y", [S, DM], F32, kind="ExternalOutput").ap()

    outer = ctx.enter_context(tc.tile_pool(name="outer", bufs=1))

    # Weights/x load as ONE batched DMA each (the sync queue costs ~600ns per
    # DMA issue, so 8 per-tile DMAs would be issue-bound, not bandwidth-bound).
    # DMA issue order tracks first use: wq + x chunk 0 + wk (Q/K proj gates
    # the first scores), then wv (V pass), x chunk 1, wo.
    def load_w8(wT, dst):
        nc.sync.dma_start(out=dst[:], in_=wT[:, :])

    tri_sb = outer.tile([128, 128], F32, tag="tri", name="tri")
    nc.sync.dma_start(out=tri_sb, in_=tri)
    wq_all = outer.tile([128, NKT * H * DK], BF16, tag="wq", name="wq_all")
    wk_all = outer.tile([128, NKT * H * DK], BF16, tag="wk", name="wk_all")
    wq = [wq_all[:, i * 512:(i + 1) * 512] for i in range(NKT)]
    wk = [wk_all[:, i * 512:(i + 1) * 512] for i in range(NKT)]
    load_w8(wqT, wq_all)

    xp = ctx.enter_context(tc.tile_pool(name="xpool", bufs=3))
    xts = {}

    def load_x(j):
        xt_all = xp.tile([128, NKT * 512], BF16, tag="xt", name="xt")
        nc.sync.dma_start(out=xt_all[:], in_=xT[:, ts(j, NKT * 512)])
        xts[j] = [xt_all[:, i * 512:(i + 1) * 512] for i in range(NKT)]

    load_x(0)

    load_w8(wkT, wk_all)
    wv_all = outer.tile([128, NKT * H * DK], BF16, tag="wv", name="wv_all")
    wv = [wv_all[:, i * 512:(i + 1) * 512] for i in range(NKT)]
    load_w8(wvT, wv_all)

    v_sb = [outer.tile([128, H * AUG], BF16, tag=f"v{t}", name=f"v{t}")
            for t in range(NTT)]
    ones1 = outer.tile([128, 1], F32, tag="ones1", name="ones1")
    nc.vector.memset(ones1[:], 1.0)
    wo_all = outer.tile([128, PAIRS * DM], BF16, tag="wo", name="wo_all")
    wo = [wo_all[:, p * DM:(p + 1) * DM] for p in range(PAIRS)]
    kT = [outer.tile([128, S], BF16, tag=f"kT{p}", name=f"kT{p}")
          for p in range(PAIRS)]

    qcp = ctx.enter_context(tc.tile_pool(name="qcp", bufs=9))
    ap_ = ctx.enter_context(tc.tile_pool(name="attn", bufs=3))
    sgp = ctx.enter_context(tc.tile_pool(name="sgp", bufs=6))
    cxp = ctx.enter_context(tc.tile_pool(name="cxp", bufs=2))
    yp = ctx.enter_context(tc.tile_pool(name="yp", bufs=3))
    ps_w = ctx.enter_context(tc.tile_pool(name="psw", bufs=2, space="PSUM"))
    ps_s = ctx.enter_context(tc.tile_pool(name="pscore", bufs=2, space="PSUM"))
    ps_o = ctx.enter_context(tc.tile_pool(name="pout", bufs=1, space="PSUM"))

    qcs = {j: [None] * PAIRS for j in range(NQC)}
    cxs = {j: [None] * PAIRS for j in range(NQC)}
    v_ready = [False] * NTT
    k_done = set()

    # ---- schedulable work units ----
    def v_unit(j, tt):
        def run():
            t = 4 * j + tt
            psv = ps_w.tile([128, 512], F32, tag="ps", name="ps")
            for i in range(NKT):
                nc.tensor.matmul(psv[:], xts[j][i][:, ts(tt, 128)], wv[i][:],
                                 start=(i == 0), stop=(i == NKT - 1))
            vt = v_sb[t]
            nc.vector.tensor_copy(
                vt[:].rearrange("p (h a) -> p h a", a=AUG)[:, :, 0:DK],
                psv[:].rearrange("p (h a) -> p h a", a=DK))
            ones_col = vt[:].rearrange("p (h a) -> p h a", a=AUG)[:, :, DK]
            nc.vector.tensor_copy(ones_col, ones1[:].to_broadcast((128, H)))
            v_ready[t] = True
        return run

    def q_unit(j, p):
        def run():
            psq = ps_w.tile([128, 512], F32, tag="ps", name="ps")
            for i in range(NKT):
                nc.tensor.matmul(psq[:], wq[i][:, ts(p, 128)], xts[j][i][:],
                                 start=(i == 0), stop=(i == NKT - 1))
            q_ = qcp.tile([128, 512], BF16, tag="qc", name="qc")
            nc.vector.tensor_copy(q_[:], psq[:])
            qcs[j][p] = q_
        return run

    def k_unit(j, p):
        def run():
            psk = ps_w.tile([128, 512], F32, tag="ps", name="ps")
            for i in range(NKT):
                nc.tensor.matmul(psk[:], wk[i][:, ts(p, 128)], xts[j][i][:],
                                 start=(i == 0), stop=(i == NKT - 1))
            nc.vector.tensor_copy(kT[p][:, ts(j, 512)], psk[:])
            k_done.add((j, p))
        return run

    def y_unit(jj, tt):
        def run():
            cxc = cxs[jj]
            t = 4 * jj + tt
            ysb = yp.tile([128, DM], F32, tag="y", name="ysb")
            for oc in range(2):
                psy = ps_w.tile([128, 512], F32, tag="ps", name="ps")
                for p in range(PAIRS):
                    nc.tensor.matmul(psy[:], cxc[p][:, ts(tt, 128)],
                                     wo[p][:, ts(oc, 512)],
                                     start=(p == 0), stop=(p == PAIRS - 1))
                nc.vector.tensor_copy(ysb[:, ts(oc, 512)], psy[:])
            nc.sync.dma_start(out=y[ts(t, 128), :], in_=ysb[:])
        return run

    # ---- filler scheduler ----
    fillers = deque()
    tail_reserve = []

    def pump(n=1):
        for _ in range(n):
            if not fillers:
                return
            fillers.popleft()()

    def pump_until(pred):
        while not pred():
            assert fillers, "filler queue exhausted before dependency met"
            pump()

    def drain_fillers():
        while fillers:
            pump()

    def prep_units(j):
        u = []
        for p in range(PAIRS):
            u.append(q_unit(j, p))
            u.append(k_unit(j, p))
            u.append(v_unit(j, p))
        return u

    # ---- softmax denominator -> normalized cx, per pair ----
    # stg holds both heads side by side ([AUG, 1024]) so one DMA moves both
    # denominator rows to DRAM for the partition-broadcast read-back.
    def pair_post(j, p, stg):
        cx = cxp.tile([128, 512], BF16, tag=f"cx{p}", name=f"cx{p}")
        g = cxp.tile([1, 1024], F32, tag="g", name="g", bufs=4)
        nc.sync.dma_start(out=g, in_=stg[DK:AUG, :])
        for hs in range(2):
            R2 = cxp.tile([64, 512], F32, tag="R2", name="R2")
            nc.gpsimd.partition_broadcast(R2[:], g[0:1, ts(hs, 512)],
                                          channels=64)
            nc.vector.reciprocal_approx_fast(out=R2[:], in_=R2[:])
            nc.vector.tensor_mul(cx[64 * hs:64 * hs + 64, :],
                                 stg[0:DK, ts(hs, 512)], R2[:])
        cxs[j][p] = cx

    # ---- attention for one chunk, fillers pumped between i-tiles ----
    def attention_chunk(j):
        nk = 4 * j + 4
        n_tiles = PAIRS * nk
        rate = len(fillers) / n_tiles if n_tiles else 0.0
        acc = 0.0
        carry = None   # (pa, pi, pc0, oa, ob, ha, hb, p)

        def _drain(c):
            pa_, pi_, pc0_, oa_, ob_, ha_, hb_, p_ = c
            nc.tensor.matmul(oa_[0:AUG, pc0_:512],
                             v_sb[pi_][:, ha_ * AUG:(ha_ + 1) * AUG],
                             pa_[:, pc0_:512], start=(pi_ == 0), stop=True)
            nc.tensor.matmul(ob_[0:AUG, pc0_:512],
                             v_sb[pi_][:, hb_ * AUG:(hb_ + 1) * AUG],
                             pa_[:, 512 + pc0_:1024], start=(pi_ == 0),
                             stop=True)
            stg = sgp.tile([AUG, 1024], F32, tag="stg", name="stg")
            nc.vector.tensor_copy(stg[:, 0:512], oa_[:])
            nc.vector.tensor_copy(stg[:, 512:1024], ob_[:])
            pair_post(j, p_, stg)

        for p in range(PAIRS):
            ha, hb = 2 * p, 2 * p + 1
            pump_until(lambda: qcs[j][p] is not None)
            pump_until(lambda: (j, p) in k_done)
            qc = qcs[j][p]
            oa = ps_o.tile([AUG, 512], F32, tag="oa", name="oa")
            ob = ps_o.tile([AUG, 512], F32, tag="ob", name="ob")
            prev = None
            for i in range(nk):
                d = i - 4 * j
                c0 = 128 * d if d > 0 else 0
                w = 512 - c0
                at = ap_.tile([128, 1024], BF16, tag="at", name="at")
                sp = ps_s.tile([128, 1024], F32, tag="sp", name="sp")
                nc.tensor.matmul(sp[0:128, c0:512],
                                 kT[p][0:64, ts(i, 128)],
                                 qc[0:64, bass.ds(c0, w)],
                                 start=True, stop=True)
                nc.tensor.matmul(sp[0:128, 512 + c0:1024],
                                 kT[p][64:128, ts(i, 128)],
                                 qc[64:128, bass.ds(c0, w)],
                                 start=True, stop=True)
                if carry is not None:
                    _drain(carry)
                    carry = None
                if c0 == 0:
                    nc.scalar.activation(at[:], sp[:],
                                         mybir.ActivationFunctionType.Exp,
                                         scale=0.125)
                else:
                    nc.scalar.activation(at[:, c0:512], sp[0:128, c0:512],
                                         mybir.ActivationFunctionType.Exp,
                                         scale=0.125)
                    nc.scalar.activation(at[:, 512 + c0:1024],
                                         sp[0:128, 512 + c0:1024],
                                         mybir.ActivationFunctionType.Exp,
                                         scale=0.125)
                if d >= 0:
                    strip = bass.AP(tensor=at.tensor, offset=at.offset + c0,
                                    ap=[list(at.ap[0]), [512, 2], [1, 128]])
                    tri_b = bass.AP(tensor=tri_sb.tensor, offset=tri_sb.offset,
                                    ap=[list(tri_sb.ap[0]), [0, 2], [1, 128]])
                    nc.vector.tensor_mul(strip, strip, tri_b)
                if prev is not None:
                    pa, pi, pc0 = prev
                    pump_until(lambda: v_ready[pi])
                    nc.tensor.matmul(oa[0:AUG, pc0:512],
                                     v_sb[pi][:, ha * AUG:(ha + 1) * AUG],
                                     pa[:, pc0:512], start=(pi == 0),
                                     stop=False)
                    nc.tensor.matmul(ob[0:AUG, pc0:512],
                                     v_sb[pi][:, hb * AUG:(hb + 1) * AUG],
                                     pa[:, 512 + pc0:1024], start=(pi == 0),
                                     stop=False)
                prev = (at, i, c0)
                acc += rate
                while acc >= 1.0 and fillers:
                    pump()
                    acc -= 1.0
            pa, pi, pc0 = prev
            pump_until(lambda: v_ready[pi])
            carry = (pa, pi, pc0, oa, ob, ha, hb, p)
        if j == NQC - 1:
            # Tail: run the reserved y(j-1) units and pre-open the first
            # output-projection psum group on pairs 0-2, so the PE stays busy
            # (and warm) while the last pair's exp backlog drains and its
            # denominator chain runs.
            for u in tail_reserve:
                u()
            ysb0 = yp.tile([128, DM], F32, tag="y", name="ysb")
            opens = []
            for oc in range(2):
                psy = ps_w.tile([128, 512], F32, tag="ps", name="ps")
                for p_ in range(PAIRS - 1):
                    nc.tensor.matmul(psy[:], cxs[j][p_][:, ts(0, 128)],
                                     wo[p_][:, ts(oc, 512)],
                                     start=(p_ == 0), stop=False)
                opens.append(psy)
        _drain(carry)
        if j == NQC - 1:
            for oc, psy in enumerate(opens):
                nc.tensor.matmul(psy[:], cxs[j][PAIRS - 1][:, ts(0, 128)],
                                 wo[PAIRS - 1][:, ts(oc, 512)],
                                 start=False, stop=True)
                nc.vector.tensor_copy(ysb0[:, ts(oc, 512)], psy[:])
            nc.sync.dma_start(out=y[ts(4 * j, 128), :], in_=ysb0[:])
            for tt in range(1, 4):
                y_unit(j, tt)()
        drain_fillers()

    # ---- prologue: QK(0) p0 inline, V(0) + remaining QK(0) as fillers ----
    q_unit(0, 0)()
    k_unit(0, 0)()
    load_x(1)
    nc.sync.dma_start(out=wo_all[:], in_=woT[:, :])
    fillers.append(v_unit(0, 0))
    for p in range(1, PAIRS):
        fillers.append(q_unit(0, p))
        fillers.append(k_unit(0, p))
        fillers.append(v_unit(0, p))
    for u in prep_units(1):
        fillers.append(u)

    attention_chunk(0)

    # ---- steady state ----
    # y(j) is emitted two chunks later (during chunk j+2) where possible:
    # the late chunks' attention is ACT(exp)-bound, so their PE slack absorbs
    # the output projections, while the early chunks stay lean.
    for j in range(1, NQC):
        if j + 1 < NQC:
            load_x(j + 1)
            prep = prep_units(j + 1)
        else:
            prep = []
        if j == 2:
            ys = [y_unit(0, tt) for tt in range(4)]
        elif j == 3:
            ys = [y_unit(1, tt) for tt in range(4)]
            ys += [y_unit(2, tt) for tt in range(2)]
            tail_reserve.extend(y_unit(2, tt) for tt in range(2, 4))
        else:
            ys = []
        mix = []
        pi_, yi_ = 0, 0
        for n in range(len(prep) + len(ys)):
            if n % 4 == 3 and yi_ < len(ys):
                mix.append(ys[yi_]); yi_ += 1
            elif pi_ < len(prep):
                mix.append(prep[pi_]); pi_ += 1
            else:
                mix.append(ys[yi_]); yi_ += 1
        fillers.extend(mix)
        attention_chunk(j)


_NC_CACHE = None


def _build():
    global _NC_CACHE
    if _NC_CACHE is None:
        from contextlib import ExitStack
        nc = bacc.Bacc("TRN2", target_bir_lowering=False, debug=False,
                       num_devices=N_CORES)
        with tile.TileContext(nc) as tc:
            with ExitStack() as ctx:
                _kernel_body(ctx, tc)
        nc.compile()
        _NC_CACHE = nc
    return _NC_CACHE


def _make_tri():
    K = np.arange(128)[:, None]
    Q = np.arange(128)[None, :]
    return (Q >= K).astype(np.float32)


def _tile8(w):
    # [1024, C] -> [128, 8*C]: row 128i+p lands at (partition p, cols i*C:..)
    n, c = w.shape
    return np.ascontiguousarray(
        w.reshape(n // 128, 128, c).transpose(1, 0, 2).reshape(128, -1))


def kernel(x, W_q, W_k, W_v, W_o, _trace=False, _tmpdir=None):
    x = np.asarray(x, dtype=np.float32)
    tri = _make_tri()
    in_maps = []
    for c in range(N_CORES):
        b, g = divmod(c, 2)
        rows = slice(512 * g, 512 * (g + 1))
        bf = ml_dtypes.bfloat16
        xc = x[b].T  # [DM, S]
        xt_host = np.concatenate(
            [_tile8(xc[:, 512 * j:512 * (j + 1)]) for j in range(NQC)], axis=1)
        in_maps.append({
            "xT": xt_host.astype(bf),
            "wqT": _tile8(np.asarray(W_q)[rows, :].T).astype(bf),
            "wkT": _tile8(np.asarray(W_k)[rows, :].T).astype(bf),
            "wvT": _tile8(np.asarray(W_v)[rows, :].T).astype(bf),
            "woT": _tile8(np.asarray(W_o)[:, rows].T).astype(bf),
            "tri": tri,
        })
    nc = _build()
    res = run_bass_kernel_spmd(nc, in_maps, core_ids=list(range(N_CORES)),
                               trace=_trace, tmpdir=_tmpdir)
    out = np.stack([res.results[2 * b]["y"] + res.results[2 * b + 1]["y"]
                    for b in range(B)]).astype(np.float32)
    kernel._last_exec_time_ns = res.exec_time_ns
    kernel._last_results = res
    return out


# revision 34
# speedup vs baseline: 1.0269x; 1.0269x over previous
"""Multi-head self-attention (b=4, s=2048, d_model=1024, h=16, causal) on 8 trn2 cores.

Sharding: core c = (batch b = c//2, head-group g = c%2): 8 heads of one batch
per core, full QKV + causal attention + partial W_o projection on device; host
pre-transposes x/W slices and sums the two partial y's per batch (the W_o
all-reduce done at unshard time).

Single software-pipelined schedule: the ACT-bound exp stream of chunk j's
attention is the backbone; the PE-bound prep work for chunk j+1 (V and Q/K
projections) and the output projection of chunk j-1 are emitted as "filler
units" between attention i-tiles so the PE fills its exp-wait gaps and never
idles long enough for the HAM clock gate to re-throttle. x is loaded once per
chunk (shared by the V and Q/K projections), weight DMAs are issued in
first-use order (wv, x0, wq/wk, x1, wo), and the softmax-denominator
reciprocal+broadcast chain runs per pair right at its drain so the final
chunk's output projection has a short tail.

Scores/projections run as float32r (full PE rate at N>=256); the attn-weight /
V side runs bf16. Attention uses the transposed layout S^T[k,q] = K @ Q^T with
the two heads of a pair row-packed via tile_position (0,0)/(64,0) (concurrent
K=64 matmuls) into one [128,1024] psum tile -> single Exp per block. V carries
an appended ones column so denominators fall out of the attn@V matmul (row
64). Causality: block skip + column restriction + one triangular strip mask.
"""

from collections import deque

import ml_dtypes
import numpy as np

import concourse.bass as bass
import concourse.tile as tile
from concourse import bacc, mybir
from concourse.bass import ts
from concourse.bass_utils import run_bass_kernel_spmd

F32 = mybir.dt.float32
F32R = mybir.dt.float32r
BF16 = mybir.dt.bfloat16

B = 4
S = 2048
DM = 1024
DK = 64
N_CORES = 8
H = 8
PAIRS = 4
NKT = DM // 128   # 8 contraction tiles
NTT = S // 128    # 16 token tiles
NQC = S // 512    # 4 query chunks
AUG = DK + 1      # 65


def _kernel_body(ctx, tc):
    nc = tc.nc
    # All inputs arrive pre-tiled on the host to [128, ...] partition-major
    # layouts so every load is a single fully-contiguous DMA.
    xT = nc.dram_tensor("xT", [128, NQC * NKT * 512], BF16,
                        kind="ExternalInput").ap()
    wqT = nc.dram_tensor("wqT", [128, NKT * H * DK], BF16,
                         kind="ExternalInput").ap()
    wkT = nc.dram_tensor("wkT", [128, NKT * H * DK], BF16,
                         kind="ExternalInput").ap()
    wvT = nc.dram_tensor("wvT", [128, NKT * H * DK], BF16,
                         kind="ExternalInput").ap()
    woT = nc.dram_tensor("woT", [128, PAIRS * DM], BF16,
                         kind="ExternalInput").ap()
    tri = nc.dram_tensor("tri", [128, 128], F32, kind="ExternalInput").ap()
    y = nc.dram_tensor("y", [S, DM], F32, kind="ExternalOutput").ap()

    outer = ctx.enter_context(tc.tile_pool(name="outer", bufs=1))

    # Weights/x load as ONE batched DMA each (the sync queue costs ~600ns per
    # DMA issue, so 8 per-tile DMAs would be issue-bound, not bandwidth-bound).
    # DMA issue order tracks first use: wq + x chunk 0 + wk (Q/K proj gates
    # the first scores), then wv (V pass), x chunk 1, wo.
    def load_w8(wT, dst):
        nc.sync.dma_start(out=dst[:], in_=wT[:, :])

    tri_sb = outer.tile([128, 128], F32, tag="tri", name="tri")
    nc.sync.dma_start(out=tri_sb, in_=tri)
    wq_all = outer.tile([128, NKT * H * DK], BF16, tag="wq", name="wq_all")
    wk_all = outer.tile([128, NKT * H * DK], BF16, tag="wk", name="wk_all")
    wq = [wq_all[:, i * 512:(i + 1) * 512] for i in range(NKT)]
    wk = [wk_all[:, i * 512:(i + 1) * 512] for i in range(NKT)]
    load_w8(wqT, wq_all)

    xp = ctx.enter_context(tc.tile_pool(name="xpool", bufs=3))
    xts = {}

    def load_x(j):
        xt_all = xp.tile([128, NKT * 512], BF16, tag="xt", name="xt")
        nc.sync.dma_start(out=xt_all[:], in_=xT[:, ts(j, NKT * 512)])
        xts[j] = [xt_all[:, i * 512:(i + 1) * 512] for i in range(NKT)]

    load_x(0)

    load_w8(wkT, wk_all)
    wv_all = outer.tile([128, NKT * H * DK], BF16, tag="wv", name="wv_all")
    wv = [wv_all[:, i * 512:(i + 1) * 512] for i in range(NKT)]
    load_w8(wvT, wv_all)

    v_sb = [outer.tile([128, H * AUG], BF16, tag=f"v{t}", name=f"v{t}")
            for t in range(NTT)]
    ones1 = outer.tile([128, 1], F32, tag="ones1", name="ones1")
    nc.vector.memset(ones1[:], 1.0)
    wo_all = outer.tile([128, PAIRS * DM], BF16, tag="wo", name="wo_all")
    wo = [wo_all[:, p * DM:(p + 1) * DM] for p in range(PAIRS)]
    kT = [outer.tile([128, S], BF16, tag=f"kT{p}", name=f"kT{p}")
          for p in range(PAIRS)]

    qcp = ctx.enter_context(tc.tile_pool(name="qcp", bufs=9))
    ap_ = ctx.enter_context(tc.tile_pool(name="attn", bufs=3))
    sgp = ctx.enter_context(tc.tile_pool(name="sgp", bufs=6))
    cxp = ctx.enter_context(tc.tile_pool(name="cxp", bufs=2))
    yp = ctx.enter_context(tc.tile_pool(name="yp", bufs=3))
    ps_w = ctx.enter_context(tc.tile_pool(name="psw", bufs=2, space="PSUM"))
    ps_s = ctx.enter_context(tc.tile_pool(name="pscore", bufs=2, space="PSUM"))
    ps_o = ctx.enter_context(tc.tile_pool(name="pout", bufs=1, space="PSUM"))

    qcs = {j: [None] * PAIRS for j in range(NQC)}
    cxs = {j: [None] * PAIRS for j in range(NQC)}
    v_ready = [False] * NTT
    k_done = set()

    # ---- schedulable work units ----
    def v_unit(j, tt):
        def run():
            t = 4 * j + tt
            psv = ps_w.tile([128, 512], F32, tag="ps", name="ps")
            for i in range(NKT):
                nc.tensor.matmul(psv[:], xts[j][i][:, ts(tt, 128)], wv[i][:],
                                 start=(i == 0), stop=(i == NKT - 1))
            vt = v_sb[t]
            nc.vector.tensor_copy(
                vt[:].rearrange("p (h a) -> p h a", a=AUG)[:, :, 0:DK],
                psv[:].rearrange("p (h a) -> p h a", a=DK))
            ones_col = vt[:].rearrange("p (h a) -> p h a", a=AUG)[:, :, DK]
            nc.vector.tensor_copy(ones_col, ones1[:].to_broadcast((128, H)))
            v_ready[t] = True
        return run

    def q_unit(j, p):
        def run():
            psq = ps_w.tile([128, 512], F32, tag="ps", name="ps")
            for i in range(NKT):
                nc.tensor.matmul(psq[:], wq[i][:, ts(p, 128)], xts[j][i][:],
                                 start=(i == 0), stop=(i == NKT - 1))
            q_ = qcp.tile([128, 512], BF16, tag="qc", name="qc")
            nc.vector.tensor_copy(q_[:], psq[:])
            qcs[j][p] = q_
        return run

    def k_unit(j, p):
        def run():
            psk = ps_w.tile([128, 512], F32, tag="ps", name="ps")
            for i in range(NKT):
                nc.tensor.matmul(psk[:], wk[i][:, ts(p, 128)], xts[j][i][:],
                                 start=(i == 0), stop=(i == NKT - 1))
            nc.vector.tensor_copy(kT[p][:, ts(j, 512)], psk[:])
            k_done.add((j, p))
        return run

    def y_unit(jj, tt):
        def run():
            cxc = cxs[jj]
            t = 4 * jj + tt
            ysb = yp.tile([128, DM], F32, tag="y", name="ysb")
            for oc in range(2):
                psy = ps_w.tile([128, 512], F32, tag="ps", name="ps")
                for p in range(PAIRS):
                    nc.tensor.matmul(psy[:], cxc[p][:, ts(tt, 128)],
                                     wo[p][:, ts(oc, 512)],
                                     start=(p == 0), stop=(p == PAIRS - 1))
                nc.vector.tensor_copy(ysb[:, ts(oc, 512)], psy[:])
            nc.sync.dma_start(out=y[ts(t, 128), :], in_=ysb[:])
        return run

    # ---- filler scheduler ----
    fillers = deque()
    tail_reserve = []

    def pump(n=1):
        for _ in range(n):
            if not fillers:
                return
            fillers.popleft()()

    def pump_until(pred):
        while not pred():
            assert fillers, "filler queue exhausted before dependency met"
            pump()

    def drain_fillers():
        while fillers:
            pump()

    def prep_units(j, with_v=True):
        u = []
        for p in range(PAIRS):
            u.append(q_unit(j, p))
            u.append(k_unit(j, p))
            if with_v:
                u.append(v_unit(j, p))
        return u

    # ---- softmax denominator -> normalized cx, per pair ----
    # stg holds both heads side by side ([AUG, 1024]) so one DMA moves both
    # denominator rows to DRAM for the partition-broadcast read-back.
    def pair_post(j, p, stg):
        cx = cxp.tile([128, 512], BF16, tag=f"cx{p}", name=f"cx{p}")
        g = cxp.tile([1, 1024], F32, tag="g", name="g", bufs=4)
        nc.sync.dma_start(out=g, in_=stg[DK:AUG, :])
        for hs in range(2):
            R2 = cxp.tile([64, 512], F32, tag="R2", name="R2")
            nc.gpsimd.partition_broadcast(R2[:], g[0:1, ts(hs, 512)],
                                          channels=64)
            nc.vector.reciprocal_approx_fast(out=R2[:], in_=R2[:])
            nc.vector.tensor_mul(cx[64 * hs:64 * hs + 64, :],
                                 stg[0:DK, ts(hs, 512)], R2[:])
        cxs[j][p] = cx

    # ---- attention for one chunk, fillers pumped between i-tiles ----
    def attention_chunk(j):
        nk = 4 * j + 4
        n_tiles = PAIRS * nk
        rate = len(fillers) / n_tiles if n_tiles else 0.0
        acc = 0.0
        carry = None   # (pa, pi, pc0, oa, ob, ha, hb, p)

        def _drain(c):
            pa_, pi_, pc0_, oa_, ob_, ha_, hb_, p_ = c
            nc.tensor.matmul(oa_[0:AUG, pc0_:512],
                             v_sb[pi_][:, ha_ * AUG:(ha_ + 1) * AUG],
                             pa_[:, pc0_:512], start=(pi_ == 0), stop=True)
            nc.tensor.matmul(ob_[0:AUG, pc0_:512],
                             v_sb[pi_][:, hb_ * AUG:(hb_ + 1) * AUG],
                             pa_[:, 512 + pc0_:1024], start=(pi_ == 0),
                             stop=True)
            stg = sgp.tile([AUG, 1024], F32, tag="stg", name="stg")
            nc.vector.tensor_copy(stg[:, 0:512], oa_[:])
            nc.vector.tensor_copy(stg[:, 512:1024], ob_[:])
            pair_post(j, p_, stg)

        for p in range(PAIRS):
            ha, hb = 2 * p, 2 * p + 1
            pump_until(lambda: qcs[j][p] is not None)
            pump_until(lambda: (j, p) in k_done)
            qc = qcs[j][p]
            oa = ps_o.tile([AUG, 512], F32, tag="oa", name="oa")
            ob = ps_o.tile([AUG, 512], F32, tag="ob", name="ob")
            prev = None
            for i in range(nk):
                d = i - 4 * j
                c0 = 128 * d if d > 0 else 0
                w = 512 - c0
                at = ap_.tile([128, 1024], BF16, tag="at", name="at")
                sp = ps_s.tile([128, 1024], F32, tag="sp", name="sp")
                nc.tensor.matmul(sp[0:128, c0:512],
                                 kT[p][0:64, ts(i, 128)],
                                 qc[0:64, bass.ds(c0, w)],
                                 start=True, stop=True)
                nc.tensor.matmul(sp[0:128, 512 + c0:1024],
                                 kT[p][64:128, ts(i, 128)],
                                 qc[64:128, bass.ds(c0, w)],
                                 start=True, stop=True)
                if carry is not None:
                    _drain(carry)
                    carry = None
                if c0 == 0:
                    nc.scalar.activation(at[:], sp[:],
                                         mybir.ActivationFunctionType.Exp,
                                         scale=0.125)
                else:
                    nc.scalar.activation(at[:, c0:512], sp[0:128, c0:512],
                                         mybir.ActivationFunctionType.Exp,
                                         scale=0.125)
                    nc.scalar.activation(at[:, 512 + c0:1024],
                                         sp[0:128, 512 + c0:1024],
                                         mybir.ActivationFunctionType.Exp,
                                         scale=0.125)
                if d >= 0:
                    strip = bass.AP(tensor=at.tensor, offset=at.offset + c0,
                                    ap=[list(at.ap[0]), [512, 2], [1, 128]])
                    tri_b = bass.AP(tensor=tri_sb.tensor, offset=tri_sb.offset,
                                    ap=[list(tri_sb.ap[0]), [0, 2], [1, 128]])
                    nc.vector.tensor_mul(strip, strip, tri_b)
                if prev is not None:
                    pa, pi, pc0 = prev
                    pump_until(lambda: v_ready[pi])
                    nc.tensor.matmul(oa[0:AUG, pc0:512],
                                     v_sb[pi][:, ha * AUG:(ha + 1) * AUG],
                                     pa[:, pc0:512], start=(pi == 0),
                                     stop=False)
                    nc.tensor.matmul(ob[0:AUG, pc0:512],
                                     v_sb[pi][:, hb * AUG:(hb + 1) * AUG],
                                     pa[:, 512 + pc0:1024], start=(pi == 0),
                                     stop=False)
                prev = (at, i, c0)
                acc += rate
                while acc >= 1.0 and fillers:
                    pump()
                    acc -= 1.0
            pa, pi, pc0 = prev
            pump_until(lambda: v_ready[pi])
            carry = (pa, pi, pc0, oa, ob, ha, hb, p)
        if j == NQC - 1:
            # Tail: run the reserved y(j-1) units and pre-open the first
            # output-projection psum group on pairs 0-2, so the PE stays busy
            # (and warm) while the last pair's exp backlog drains and its
            # denominator chain runs.
            for u in tail_reserve:
                u()
            ysb0 = yp.tile([128, DM], F32, tag="y", name="ysb")
            opens = []
            for oc in range(2):
                psy = ps_w.tile([128, 512], F32, tag="ps", name="ps")
                for p_ in range(PAIRS - 1):
                    nc.tensor.matmul(psy[:], cxs[j][p_][:, ts(0, 128)],
                                     wo[p_][:, ts(oc, 512)],
                                     start=(p_ == 0), stop=False)
                opens.append(psy)
        _drain(carry)
        if j == NQC - 1:
            for oc, psy in enumerate(opens):
                nc.tensor.matmul(psy[:], cxs[j][PAIRS - 1][:, ts(0, 128)],
                                 wo[PAIRS - 1][:, ts(oc, 512)],
                                 start=False, stop=True)
                nc.vector.tensor_copy(ysb0[:, ts(oc, 512)], psy[:])
            nc.sync.dma_start(out=y[ts(4 * j, 128), :], in_=ysb0[:])
            for tt in range(1, 4):
                y_unit(j, tt)()
        drain_fillers()

    # ---- prologue: QK(0) p0 inline, V(0) + remaining QK(0) as fillers ----
    q_unit(0, 0)()
    k_unit(0, 0)()
    load_x(1)
    nc.sync.dma_start(out=wo_all[:], in_=woT[:, :])
    fillers.append(v_unit(0, 0))
    for p in range(1, PAIRS):
        fillers.append(q_unit(0, p))
        fillers.append(k_unit(0, p))
        fillers.append(v_unit(0, p))
    for u in prep_units(1):
        fillers.append(u)

    attention_chunk(0)

    # ---- steady state ----
    # y(j) is emitted two chunks later (during chunk j+2) where possible:
    # the late chunks' attention is ACT(exp)-bound, so their PE slack absorbs
    # the output projections, while the early chunks stay lean.
    for j in range(1, NQC):
        if j + 1 < NQC:
            load_x(j + 1)
            # V(j+1) is only consumed midway through chunk j+1's attention,
            # so for the last chunk (which has ACT-bound PE slack) the V
            # units move into that chunk instead of the PE-bound chunk j.
            prep = prep_units(j + 1, with_v=(j + 1 < NQC - 1))
        else:
            prep = [v_unit(j, tt) for tt in range(4)]
        if j == 2:
            ys = [y_unit(0, tt) for tt in range(4)]
        elif j == 3:
            ys = [y_unit(1, tt) for tt in range(4)]
            ys += [y_unit(2, tt) for tt in range(2)]
            tail_reserve.extend(y_unit(2, tt) for tt in range(2, 4))
        else:
            ys = []
        mix = []
        pi_, yi_ = 0, 0
        for n in range(len(prep) + len(ys)):
            if n % 4 == 3 and yi_ < len(ys):
                mix.append(ys[yi_]); yi_ += 1
            elif pi_ < len(prep):
                mix.append(prep[pi_]); pi_ += 1
            else:
                mix.append(ys[yi_]); yi_ += 1
        fillers.extend(mix)
        attention_chunk(j)


_NC_CACHE = None


def _build():
    global _NC_CACHE
    if _NC_CACHE is None:
        from contextlib import ExitStack
        nc = bacc.Bacc("TRN2", target_bir_lowering=False, debug=False,
                       num_devices=N_CORES)
        with tile.TileContext(nc) as tc:
            with ExitStack() as ctx:
                _kernel_body(ctx, tc)
        nc.compile()
        _NC_CACHE = nc
    return _NC_CACHE


def _make_tri():
    K = np.arange(128)[:, None]
    Q = np.arange(128)[None, :]
    return (Q >= K).astype(np.float32)


def _tile8(w):
    # [1024, C] -> [128, 8*C]: row 128i+p lands at (partition p, cols i*C:..)
    n, c = w.shape
    return np.ascontiguousarray(
        w.reshape(n // 128, 128, c).transpose(1, 0, 2).reshape(128, -1))


def kernel(x, W_q, W_k, W_v, W_o, _trace=False, _tmpdir=None):
    x = np.asarray(x, dtype=np.float32)
    tri = _make_tri()
    in_maps = []
    for c in range(N_CORES):
        b, g = divmod(c, 2)
        rows = slice(512 * g, 512 * (g + 1))
        bf = ml_dtypes.bfloat16
        xc = x[b].T  # [DM, S]
        xt_host = np.concatenate(
            [_tile8(xc[:, 512 * j:512 * (j + 1)]) for j in range(NQC)], axis=1)
        in_maps.append({
            "xT": xt_host.astype(bf),
            "wqT": _tile8(np.asarray(W_q)[rows, :].T).astype(bf),
            "wkT": _tile8(np.asarray(W_k)[rows, :].T).astype(bf),
            "wvT": _tile8(np.asarray(W_v)[rows, :].T).astype(bf),
            "woT": _tile8(np.asarray(W_o)[:, rows].T).astype(bf),
            "tri": tri,
        })
    nc = _build()
    res = run_bass_kernel_spmd(nc, in_maps, core_ids=list(range(N_CORES)),
                               trace=_trace, tmpdir=_tmpdir)
    out = np.stack([res.results[2 * b]["y"] + res.results[2 * b + 1]["y"]
                    for b in range(B)]).astype(np.float32)
    kernel._last_exec_time_ns = res.exec_time_ns
    kernel._last_results = res
    return out
